# revision 13
# baseline (speedup 1.0000x reference)
"""Distributed Trainium2 kernel for the dense_transformer attention block.

Sharding: 16 heads / 8 cores = 2 heads per core (tensor parallel), AllToAll
token-exchange before the output projection, each core computes a 512-token
slice of the final output.

Per-core pipeline (all matmuls bf16, fp32 PSUM accumulation):
  1. qkv^T projection: feats-major  qkvT[128(h0|h1), {q,k,v}, 4096 tok]
     (q columns pre-scaled by 1/sqrt(d) on host)
  2. DMA-transpose v,k tiles to token-major V_aug [128, bh, kt, 65]
     (65th col = ones -> rowsum of exp rides the V matmul for free)
  3. KtV = K^T V  [64,64] per (b,h)  (rank-64 shortcut for the 0.01*S term
     of leaky_relu: leaky(S)@V = 0.99*relu(S)@V + 0.01*S@V,
     S@V = Q@(K^T V))
  4. attention per (b, h, q-chunk 512):
       S^T[kpos, q] = kT.T @ qT   (K=64 contraction, heads at partition
       bases 0/64 -> disjoint PE row groups)
       exp on ACT  [128,1024] psum->sbuf bf16
       relu on DVE [128,1024] psum->sbuf bf16
       o1[65,512]  += V_aug.T @ expS^T   (row 64 = rowsum)
       o2[64,512]  += V.T @ reluS^T ;  += (0.01/0.99*KtV).T @ qT
       epilogue: rcp = 1/rowsum; bcast via 1x64 matmul with W1 folded in;
       attnT[d, bh, s] = o1*bcast + 0.99*W2*o2
  5. AllToAll(token chunks) -> fc_out: out[512,1024] = sum_heads
     attnT.T @ Wout_rows + bout
"""

import sys

for _p in ("/opt/trn_rl_repo",):
    if _p not in sys.path:
        sys.path.insert(0, _p)

import numpy as np
import ml_dtypes

BF16 = ml_dtypes.bfloat16

E = 1024
T = 4096  # B*S
S = 2048
B = 2
D = 64
NCORES = 8
KCORR = 0.01 / 0.99

LAST_EXEC_NS = None

_NC = None


def _build(debug_taps=False):
    import concourse.bass as bass  # noqa: F401
    import concourse.mybir as mybir
    import concourse.tile as tile
    from concourse import bacc

    bf = mybir.dt.bfloat16
    f32 = mybir.dt.float32
    AF = mybir.ActivationFunctionType

    nc = bacc.Bacc(
        "TRN2",
        target_bir_lowering=False,
        debug=False,
        num_devices=NCORES,
    )

    xT_d = nc.dram_tensor("xT", [E, T], bf, kind="ExternalInput")
    wqkv_d = nc.dram_tensor("wqkv", [E, 384], bf, kind="ExternalInput")
    wout_d = nc.dram_tensor("wout", [E, E], bf, kind="ExternalInput")
    boutr_d = nc.dram_tensor("boutr", [128, E], f32, kind="ExternalInput")
    w1col_d = nc.dram_tensor("w1col", [1, 128], f32, kind="ExternalInput")
    w2rep_d = nc.dram_tensor("w2rep", [64, 2], f32, kind="ExternalInput")
    out_d = nc.dram_tensor("out", [512, E], f32, kind="ExternalOutput")
    a2a_in = nc.dram_tensor("a2a_in", [8, 64, 2, 512], bf)
    a2a_out = nc.dram_tensor("a2a_out", [8, 64, 2, 512], bf)

    taps = {}
    if debug_taps:
        taps["qkvT"] = nc.dram_tensor("dbg_qkvT", [128, 3, T], bf, kind="ExternalOutput")
        taps["vaug"] = nc.dram_tensor("dbg_vaug", [128, 4, 16, 128], bf, kind="ExternalOutput")
        taps["ktok"] = nc.dram_tensor("dbg_ktok", [128, 4, 16, 64], bf, kind="ExternalOutput")
        taps["ktvs"] = nc.dram_tensor("dbg_ktvs", [128, 2, 64], bf, kind="ExternalOutput")
        taps["attnT"] = nc.dram_tensor("dbg_attnT", [64, 4, S], bf, kind="ExternalOutput")
        taps["a2a"] = nc.dram_tensor("dbg_a2a", [8, 64, 2, 512], bf, kind="ExternalOutput")

    with tile.TileContext(nc) as tc:
        with (
            tc.tile_pool(name="const", bufs=1) as cpool,
            tc.tile_pool(name="big", bufs=1) as bigpool,
            tc.tile_pool(name="xin", bufs=3) as xpool,
            tc.tile_pool(name="exr", bufs=3) as expool,
            tc.tile_pool(name="ep", bufs=2) as eppool,
            tc.tile_pool(name="gin", bufs=3) as gpool,
            tc.tile_pool(name="osb", bufs=3) as opool,
        ):
            # ---- constants / persistent tensors ----
            wqkv_sb = cpool.tile([128, 8, 384], bf)
            nc.sync.dma_start(
                out=wqkv_sb[:],
                in_=wqkv_d.ap().rearrange("(kt p) f -> p kt f", p=128),
            )
            # Wout held 64-row-major so fc rhs slices sit at partition base 0
            wout_sb = cpool.tile([64, 16, E], bf)
            nc.sync.dma_start(
                out=wout_sb[:],
                in_=wout_d.ap().rearrange("(rt p) e -> p rt e", p=64),
            )
            boutr_sb = cpool.tile([128, E], f32)
            nc.sync.dma_start(out=boutr_sb[:], in_=boutr_d.ap())
            # W1 staged at partition 64 so the rowsum epilogue (whose data
            # sits at psum partition 64) never partition-shifts
            w1_sb = cpool.tile([65, 128], f32)
            nc.sync.dma_start(out=w1_sb[64:65, :], in_=w1col_d.ap())
            w2_sb = cpool.tile([64, 2], f32)
            nc.sync.dma_start(out=w2_sb[:], in_=w2rep_d.ap())

            qkvT = bigpool.tile([128, 3, T], bf)     # [feat(h0|h1), m, tok]
            # inner dim padded to 128 so each kt-tile's destination is
            # 256B-aligned (the DMA-transpose xbar corrupts unaligned dests);
            # col 64 = ones, cols 65-127 unused
            vaug = bigpool.tile([128, 4, 16, 128], bf)  # [kp, bh, kt, d|1|pad]
            ktok = bigpool.tile([128, 4, 16, 64], bf)  # [kp, bh, kt, d]
            attnT = bigpool.tile([64, 4, S], bf)     # [d, bh, s]
            ktvs2 = bigpool.tile([128, 2, 64], bf)   # [di(h0|h1), b, do]

            nc.vector.memset(vaug[:, :, :, 64:65], 1.0)

            # ---- phase 1: qkv^T projection ----
            with tc.tile_pool(name="pp", bufs=2, space="PSUM") as ppool:
                for tck in range(8):
                    ps = [
                        ppool.tile([128, 512], f32, tag=f"pm{m}", name=f"pm{m}")
                        for m in range(3)
                    ]
                    for kt in range(8):
                        xt = xpool.tile([128, 512], bf, tag="xt")
                        nc.sync.dma_start(
                            out=xt[:],
                            in_=xT_d.ap()[
                                kt * 128 : (kt + 1) * 128,
                                tck * 512 : (tck + 1) * 512,
                            ],
                        )
                        for m in range(3):
                            nc.tensor.matmul(
                                ps[m][:],
                                lhsT=wqkv_sb[:, kt, m * 128 : (m + 1) * 128],
                                rhs=xt[:],
                                start=(kt == 0),
                                stop=(kt == 7),
                            )
                    for m in range(3):
                        nc.any.tensor_copy(
                            out=qkvT[:, m, tck * 512 : (tck + 1) * 512],
                            in_=ps[m][:],
                        )

            # ---- phase 2: transpose v,k to token-major ----
            for b in range(2):
                for h in range(2):
                    bh, hb = b * 2 + h, h * 64
                    for kt in range(16):
                        tcol = (b * 16 + kt) * 128
                        nc.sync.dma_start(
                            out=vaug[:, bh, kt, 0:64],
                            in_=qkvT[hb : hb + 64, 2, tcol : tcol + 128],
                            transpose=True,
                        )
                        nc.sync.dma_start(
                            out=ktok[:, bh, kt, :],
                            in_=qkvT[hb : hb + 64, 1, tcol : tcol + 128],
                            transpose=True,
                        )

            # ---- phase 3: KtV rank-64 shortcut ----
            with tc.tile_pool(name="ktvp", bufs=2, space="PSUM") as ktvpool:
                for b in range(2):
                    for h in range(2):
                        bh = b * 2 + h
                        kp = ktvpool.tile([64, 64], f32, tag="ktv")
                        for kt in range(16):
                            nc.tensor.matmul(
                                kp[:],
                                lhsT=ktok[:, bh, kt, :],
                                rhs=vaug[:, bh, kt, 0:64],
                                start=(kt == 0),
                                stop=(kt == 15),
                            )
                        if h == 0:
                            nc.vector.tensor_scalar_mul(
                                out=ktvs2[0:64, b, :], in0=kp[:], scalar1=KCORR
                            )
                        else:
                            ktmp = eppool.tile([64, 64], bf, tag="ktmp")
                            nc.vector.tensor_scalar_mul(
                                out=ktmp[:], in0=kp[:], scalar1=KCORR
                            )
                            nc.sync.dma_start(
                                out=ktvs2[64:128, b, :], in_=ktmp[:]
                            )

            # ---- phase 4: attention ----
            with (
                tc.tile_pool(name="sgp", bufs=2, space="PSUM") as sgpool,
                tc.tile_pool(name="o1p", bufs=1, space="PSUM") as o1pool,
                tc.tile_pool(name="o2p", bufs=1, space="PSUM") as o2pool,
                tc.tile_pool(name="pbp", bufs=1, space="PSUM") as pbpool,
            ):
                for b in range(2):
                    for h in range(2):
                        bh, hb = b * 2 + h, h * 64
                        for qc in range(4):
                            qs = b * S + qc * 512
                            o1 = o1pool.tile([65, 512], f32, tag="o1")
                            o2 = o2pool.tile([64, 512], f32, tag="o2")
                            for ktg in range(8):
                                sg = sgpool.tile([128, 1024], f32, tag="sg")
                                for j in range(2):
                                    kt = ktg * 2 + j
                                    kcol = (b * 16 + kt) * 128
                                    nc.tensor.matmul(
                                        sg[:, j * 512 : (j + 1) * 512],
                                        lhsT=qkvT[hb : hb + 64, 1, kcol : kcol + 128],
                                        rhs=qkvT[hb : hb + 64, 0, qs : qs + 512],
                                        start=True,
                                        stop=True,
                                    )
                                ex = expool.tile([128, 1024], bf, tag="ex")
                                rl = expool.tile([128, 1024], bf, tag="rl")
                                nc.scalar.activation(
                                    out=ex[:], in_=sg[:], func=AF.Exp
                                )
                                nc.vector.tensor_scalar_max(
                                    out=rl[:], in0=sg[:], scalar1=0.0
                                )
                                for j in range(2):
                                    kt = ktg * 2 + j
                                    nc.tensor.matmul(
                                        o1[:],
                                        lhsT=vaug[:, bh, kt, 0:65],
                                        rhs=ex[:, j * 512 : (j + 1) * 512],
                                        start=(kt == 0),
                                        stop=(kt == 15),
                                    )
                                    nc.tensor.matmul(
                                        o2[:],
                                        lhsT=vaug[:, bh, kt, 0:64],
                                        rhs=rl[:, j * 512 : (j + 1) * 512],
                                        start=(kt == 0),
                                        stop=False,
                                    )
                            # rank-64 leaky correction accumulates into o2
                            nc.tensor.matmul(
                                o2[:],
                                lhsT=ktvs2[hb : hb + 64, b, :],
                                rhs=qkvT[hb : hb + 64, 0, qs : qs + 512],
                                start=False,
                                stop=True,
                            )
                            # epilogue
                            rc = eppool.tile([65, 512], f32, tag="rc")
                            nc.vector.reciprocal(
                                out=rc[64:65, :], in_=o1[64:65, :]
                            )
                            pb = pbpool.tile([64, 512], f32, tag="pb")
                            nc.tensor.matmul(
                                pb[:],
                                lhsT=w1_sb[64:65, hb : hb + 64],
                                rhs=rc[64:65, :],
                                start=True,
                                stop=True,
                            )
                            pbs = eppool.tile([64, 512], bf, tag="pbs")
                            nc.vector.tensor_copy(out=pbs[:], in_=pb[:])
                            t2 = eppool.tile([64, 512], bf, tag="t2")
                            nc.vector.tensor_scalar_mul(
                                out=t2[:], in0=o2[:], scalar1=w2_sb[:, h : h + 1]
                            )
                            t1 = eppool.tile([64, 512], bf, tag="t1")
                            nc.vector.tensor_mul(
                                out=t1[:], in0=o1[0:64, :], in1=pbs[:]
                            )
                            nc.vector.tensor_add(
                                out=attnT[:, bh, qc * 512 : (qc + 1) * 512],
                                in0=t1[:],
                                in1=t2[:],
                            )

            # ---- phase 5: AllToAll + fc_out ----
            for jc in range(8):
                bb, qq = jc // 4, jc % 4
                nc.sync.dma_start(
                    out=a2a_in.ap()[jc],
                    in_=attnT[:, bb * 2 : bb * 2 + 2, qq * 512 : (qq + 1) * 512],
                )
            nc.gpsimd.collective_compute(
                "AllToAll",
                mybir.AluOpType.bypass,
                replica_groups=[list(range(NCORES))],
                ins=[a2a_in.ap().opt()],
                outs=[a2a_out.ap().opt()],
            )
            with tc.tile_pool(name="fcp", bufs=1, space="PSUM") as fcpool:
                fps = {}
                for m in range(4):
                    for n in range(2):
                        fps[m, n] = fcpool.tile(
                            [128, 512], f32, tag=f"fc{m}{n}", name=f"fc{m}{n}"
                        )
                for blk in range(8):
                    g = gpool.tile([64, 2, 512], bf, tag="g")
                    nc.sync.dma_start(out=g[:], in_=a2a_out.ap()[blk])
                    for hh in range(2):
                        for m in range(4):
                            for n in range(2):
                                nc.tensor.matmul(
                                    fps[m, n][:],
                                    lhsT=g[:, hh, m * 128 : (m + 1) * 128],
                                    rhs=wout_sb[
                                        :, 2 * blk + hh, n * 512 : (n + 1) * 512
                                    ],
                                    start=(blk == 0 and hh == 0),
                                    stop=(blk == 7 and hh == 1),
                                )
                if debug_taps:
                    nc.sync.dma_start(out=taps["qkvT"].ap(), in_=qkvT[:])
                    nc.sync.dma_start(out=taps["vaug"].ap(), in_=vaug[:])
                    nc.sync.dma_start(out=taps["ktok"].ap(), in_=ktok[:])
                    nc.sync.dma_start(out=taps["ktvs"].ap(), in_=ktvs2[:])
                    nc.sync.dma_start(out=taps["attnT"].ap(), in_=attnT[:])
                    nc.sync.dma_start(out=taps["a2a"].ap(), in_=a2a_out.ap())
                for m in range(4):
                    for n in range(2):
                        ob = opool.tile([128, 512], f32, tag="ob")
                        nc.vector.tensor_add(
                            out=ob[:],
                            in0=fps[m, n][:],
                            in1=boutr_sb[:, n * 512 : (n + 1) * 512],
                        )
                        nc.sync.dma_start(
                            out=out_d.ap()[
                                m * 128 : (m + 1) * 128, n * 512 : (n + 1) * 512
                            ],
                            in_=ob[:],
                        )

    nc.compile()
    return nc


def _get_nc():
    global _NC
    if _NC is None:
        _NC = _build()
    return _NC


def _install_ntff_hook():
    """Provide antenv.axon_hooks (absent on this image) so trace=True can
    drive NTFF profiling through the injected libaxon_pjrt.so."""
    import types

    try:
        from antenv.axon_hooks import get_axon_ntff_profile_hook  # noqa: F401

        return
    except ImportError:
        pass
    try:
        import antenv
        from trn_agent_boot.trn_boot import _ntff_profile_via_ctypes

        mod = types.ModuleType("antenv.axon_hooks")
        mod._hook = _ntff_profile_via_ctypes("/opt/axon/libaxon_pjrt.so")
        mod.set_axon_ntff_profile_hook = lambda h: setattr(mod, "_hook", h)
        mod.get_axon_ntff_profile_hook = lambda: mod._hook
        sys.modules["antenv.axon_hooks"] = mod
        antenv.axon_hooks = mod
    except Exception:
        pass


def kernel(x, Wqkv, W1, W2, Wout, bout):
    from concourse import bass_utils
    from concourse.bass_utils import run_bass_kernel_spmd

    global LAST_EXEC_NS

    _install_ntff_hook()
    # artifact upload needs a bucket this sandbox doesn't have
    bass_utils.upload_artifacts = lambda tmpdir: tmpdir

    nc = _get_nc()

    x2 = np.asarray(x, np.float32).reshape(T, E)
    xT = np.ascontiguousarray(x2.T).astype(BF16)
    Wq = np.asarray(Wqkv, np.float32)
    wout_b = np.ascontiguousarray(np.asarray(Wout, np.float32)).astype(BF16)
    boutr = np.ascontiguousarray(
        np.broadcast_to(np.asarray(bout, np.float32), (128, E))
    )
    W1f = np.asarray(W1, np.float32).reshape(16)
    W2f = np.asarray(W2, np.float32).reshape(16)

    in_maps = []
    for c in range(NCORES):
        cols = []
        for off, scale in ((0, 0.125), (E, 1.0), (2 * E, 1.0)):
            for h in range(2):
                gh = 2 * c + h
                cols.append(Wq[:, off + gh * 64 : off + (gh + 1) * 64] * scale)
        wqkv_c = np.ascontiguousarray(np.concatenate(cols, axis=1)).astype(BF16)
        w1col = np.zeros((1, 128), np.float32)
        w1col[0, 0:64] = W1f[2 * c]
        w1col[0, 64:128] = W1f[2 * c + 1]
        w2rep = np.zeros((64, 2), np.float32)
        w2rep[:, 0] = 0.99 * W2f[2 * c]
        w2rep[:, 1] = 0.99 * W2f[2 * c + 1]
        in_maps.append(
            {
                "xT": xT,
                "wqkv": wqkv_c,
                "wout": wout_b,
                "boutr": boutr,
                "w1col": w1col,
                "w2rep": w2rep,
            }
        )

    res = run_bass_kernel_spmd(nc, in_maps, core_ids=list(range(NCORES)), trace=True)
    LAST_EXEC_NS = res.exec_time_ns

    out = np.concatenate(
        [np.asarray(res.results[c]["out"], np.float32) for c in range(NCORES)],
        axis=0,
    )
    return out.reshape(B, S, E)


# revision 19
# speedup vs baseline: 1.3883x; 1.3883x over previous
"""Distributed Trainium2 kernel for the dense_transformer attention block.

Sharding: 16 heads / 8 cores = 2 heads per core (tensor parallel), AllToAll
token-exchange before the output projection, each core computes a 512-token
slice of the final output.

Per-core pipeline (all matmuls bf16, fp32 PSUM accumulation):
  1. qkv^T projection: feats-major  qkvT[128(h0|h1), {q,k,v}, 4096 tok]
     (q columns pre-scaled by 1/sqrt(d) on host)
  2. DMA-transpose v,k tiles to token-major V_aug [128, bh, kt, 64|inv(W1)]
     (col 64 = 1/W1[h] -> the exp@V matmul's 65th row becomes rowsum/W1,
     so its fast-reciprocal is already W1/rowsum)
  3. KtV = K^T V  [64,64] per (b,h)  (rank-64 shortcut for the 0.01*S term
     of leaky_relu: leaky(S)@V = 0.99*relu(S)@V + 0.01*S@V = ... Q@(K^T V))
  4. attention per (b, q-chunk 512), heads packed in PE row groups 0/64:
       S^T[kpos, q(h0|h1)] -> one [128,1024] psum tile per kpos-tile
       exp on ACT, relu on DVE (both heads in one op)
       o1_h[65,512] += V_aug.T @ expS^T   (row 64 = rowsum/W1)
       o2_h[64,512] += V.T @ reluS^T ;  += (0.01/0.99*KtV).T @ qT
       epilogue: rc = recip_fast(o1[64]); gpsimd partition-broadcast;
       attnT[d, bh, s] = o1*rc_bcast + (0.99*W2)*o2
  5. AllToAll(token chunks) -> fc_out: out[512,1024] = sum_heads
     attnT.T @ Wout_rows + bout
"""

import sys

for _p in ("/opt/trn_rl_repo",):
    if _p not in sys.path:
        sys.path.insert(0, _p)

import numpy as np
import ml_dtypes

BF16 = ml_dtypes.bfloat16

E = 1024
T = 4096  # B*S
S = 2048
B = 2
D = 64
NCORES = 8
KCORR = 0.01 / 0.99

LAST_EXEC_NS = None

_NC = None


def _build(debug_taps=False):
    import concourse.bass as bass  # noqa: F401
    import concourse.mybir as mybir
    import concourse.tile as tile
    from concourse import bacc

    bf = mybir.dt.bfloat16
    f32 = mybir.dt.float32
    AF = mybir.ActivationFunctionType

    nc = bacc.Bacc(
        "TRN2",
        target_bir_lowering=False,
        debug=False,
        num_devices=NCORES,
    )

    xT_d = nc.dram_tensor("xT", [E, T], bf, kind="ExternalInput")
    wqkv_d = nc.dram_tensor("wqkv", [E, 384], bf, kind="ExternalInput")
    wout_d = nc.dram_tensor("wout", [E, E], bf, kind="ExternalInput")
    boutr_d = nc.dram_tensor("boutr", [128, E], f32, kind="ExternalInput")
    w1inv_d = nc.dram_tensor("w1inv", [128, 4, 16], bf, kind="ExternalInput")
    w2rep_d = nc.dram_tensor("w2rep", [64, 2], f32, kind="ExternalInput")
    out_d = nc.dram_tensor("out", [512, E], f32, kind="ExternalOutput")
    a2a_in = nc.dram_tensor("a2a_in", [8, 64, 2, 512], bf)
    a2a_out = nc.dram_tensor("a2a_out", [8, 64, 2, 512], bf)

    taps = {}
    if debug_taps:
        taps["qkvT"] = nc.dram_tensor("dbg_qkvT", [128, 3, T], bf, kind="ExternalOutput")
        taps["vaug"] = nc.dram_tensor("dbg_vaug", [128, 4, 16, 128], bf, kind="ExternalOutput")
        taps["ktok"] = nc.dram_tensor("dbg_ktok", [128, 4, 16, 64], bf, kind="ExternalOutput")
        taps["ktvs"] = nc.dram_tensor("dbg_ktvs", [128, 2, 64], bf, kind="ExternalOutput")
        taps["attnT"] = nc.dram_tensor("dbg_attnT", [64, 4, S], bf, kind="ExternalOutput")
        taps["a2a"] = nc.dram_tensor("dbg_a2a", [8, 64, 2, 512], bf, kind="ExternalOutput")
        taps["ex"] = nc.dram_tensor("dbg_ex", [128, 1024], bf, kind="ExternalOutput")
        taps["rl"] = nc.dram_tensor("dbg_rl", [128, 1024], bf, kind="ExternalOutput")
        taps["rc"] = nc.dram_tensor("dbg_rc", [65, 512], f32, kind="ExternalOutput")
        taps["pbs"] = nc.dram_tensor("dbg_pbs", [64, 512], f32, kind="ExternalOutput")
        taps["t1"] = nc.dram_tensor("dbg_t1", [64, 512], bf, kind="ExternalOutput")
        taps["t2"] = nc.dram_tensor("dbg_t2", [64, 512], bf, kind="ExternalOutput")

    with tile.TileContext(nc) as tc:
        with (
            tc.tile_pool(name="const", bufs=1) as cpool,
            tc.tile_pool(name="big", bufs=1) as bigpool,
            tc.tile_pool(name="xin", bufs=4) as xpool,
            tc.tile_pool(name="exr", bufs=3) as expool,
            tc.tile_pool(name="ep", bufs=2) as eppool,
            tc.tile_pool(name="gin", bufs=3) as gpool,
            tc.tile_pool(name="osb", bufs=3) as opool,
        ):
            # ---- constants / persistent tensors ----
            wqkv_sb = cpool.tile([128, 8, 384], bf)
            nc.sync.dma_start(
                out=wqkv_sb[:],
                in_=wqkv_d.ap().rearrange("(kt p) f -> p kt f", p=128),
            )
            wout_sb = cpool.tile([128, 8, E], bf)
            nc.sync.dma_start(
                out=wout_sb[:],
                in_=wout_d.ap().rearrange("(rt p) e -> p rt e", p=128),
            )
            boutr_sb = cpool.tile([128, E], f32)
            nc.sync.dma_start(out=boutr_sb[:], in_=boutr_d.ap())
            w2_sb = cpool.tile([64, 2], f32)
            nc.sync.dma_start(out=w2_sb[:], in_=w2rep_d.ap())

            qkvT = bigpool.tile([128, 3, T], bf)     # [feat(h0|h1), m, tok]
            # inner dim padded to 128 so each kt-tile's transpose destination
            # is 256B-aligned (unaligned DMA-transpose dests corrupt);
            # col 64 = 1/W1[h], cols 65..127 unused
            vaug = bigpool.tile([128, 4, 16, 128], bf)  # [kp, bh, kt, d|w1inv|pad]
            ktok = bigpool.tile([128, 4, 16, 64], bf)  # [kp, bh, kt, d]
            attnT = bigpool.tile([64, 4, S], bf)     # [d, bh, s]
            ktvs2 = bigpool.tile([128, 2, 64], bf)   # [di(h0|h1), b, do]

            nc.sync.dma_start(out=vaug[:, :, :, 64:65], in_=w1inv_d.ap())

            # ---- phase 1: qkv^T projection ----
            with tc.tile_pool(name="pp", bufs=2, space="PSUM") as ppool:
                for tck in range(8):
                    ps = [
                        ppool.tile([128, 512], f32, tag=f"pm{m}", name=f"pm{m}")
                        for m in range(3)
                    ]
                    for kt in range(8):
                        xt = xpool.tile([128, 512], bf, tag="xt")
                        nc.sync.dma_start(
                            out=xt[:],
                            in_=xT_d.ap()[
                                kt * 128 : (kt + 1) * 128,
                                tck * 512 : (tck + 1) * 512,
                            ],
                        )
                        for m in range(3):
                            nc.tensor.matmul(
                                ps[m][:],
                                lhsT=wqkv_sb[:, kt, m * 128 : (m + 1) * 128],
                                rhs=xt[:],
                                start=(kt == 0),
                                stop=(kt == 7),
                            )
                    for m in range(3):
                        nc.any.tensor_copy(
                            out=qkvT[:, m, tck * 512 : (tck + 1) * 512],
                            in_=ps[m][:],
                        )

            # ---- phase 2: transpose v,k to token-major (scalar HWDGE queue) ----
            for b in range(2):
                for h in range(2):
                    bh, hb = b * 2 + h, h * 64
                    for kt in range(16):
                        tcol = (b * 16 + kt) * 128
                        nc.scalar.dma_start(
                            out=vaug[:, bh, kt, 0:64],
                            in_=qkvT[hb : hb + 64, 2, tcol : tcol + 128],
                            transpose=True,
                        )
                        nc.scalar.dma_start(
                            out=ktok[:, bh, kt, :],
                            in_=qkvT[hb : hb + 64, 1, tcol : tcol + 128],
                            transpose=True,
                        )

            # ---- phase 3: KtV rank-64 shortcut ----
            with tc.tile_pool(name="ktvp", bufs=2, space="PSUM") as ktvpool:
                for b in range(2):
                    for h in range(2):
                        bh = b * 2 + h
                        kp = ktvpool.tile([64, 64], f32, tag="ktv")
                        for kt in range(16):
                            nc.tensor.matmul(
                                kp[:],
                                lhsT=ktok[:, bh, kt, :],
                                rhs=vaug[:, bh, kt, 0:64],
                                start=(kt == 0),
                                stop=(kt == 15),
                            )
                        if h == 0:
                            nc.vector.tensor_scalar_mul(
                                out=ktvs2[0:64, b, :], in0=kp[:], scalar1=KCORR
                            )
                        else:
                            ktmp = eppool.tile([64, 64], bf, tag="ktmp")
                            nc.vector.tensor_scalar_mul(
                                out=ktmp[:], in0=kp[:], scalar1=KCORR
                            )
                            nc.sync.dma_start(
                                out=ktvs2[64:128, b, :], in_=ktmp[:]
                            )

            # ---- phase 4: attention (heads packed in PE row groups) ----
            with (
                tc.tile_pool(name="sgp", bufs=2, space="PSUM") as sgpool,
                tc.tile_pool(name="o1p", bufs=1, space="PSUM") as o1pool,
                tc.tile_pool(name="o2p", bufs=1, space="PSUM") as o2pool,
            ):
                for b in range(2):
                    for qc in range(4):
                        qs = b * S + qc * 512
                        o1 = [
                            o1pool.tile([65, 512], f32, tag=f"o1h{h}", name=f"o1h{h}")
                            for h in range(2)
                        ]
                        o2 = [
                            o2pool.tile([64, 512], f32, tag=f"o2h{h}", name=f"o2h{h}")
                            for h in range(2)
                        ]
                        for kt in range(16):
                            kcol = (b * 16 + kt) * 128
                            sg = sgpool.tile([128, 1024], f32, tag="sg")
                            for h in range(2):
                                hb = h * 64
                                nc.tensor.matmul(
                                    sg[:, h * 512 : (h + 1) * 512],
                                    lhsT=qkvT[hb : hb + 64, 1, kcol : kcol + 128],
                                    rhs=qkvT[hb : hb + 64, 0, qs : qs + 512],
                                    start=True,
                                    stop=True,
                                )
                            ex = expool.tile([128, 1024], bf, tag="ex")
                            rl = expool.tile([128, 1024], bf, tag="rl")
                            nc.scalar.activation(out=ex[:], in_=sg[:], func=AF.Exp)
                            nc.vector.tensor_scalar_max(
                                out=rl[:], in0=sg[:], scalar1=0.0
                            )
                            if debug_taps and b == 0 and qc == 0 and kt == 0:
                                nc.sync.dma_start(out=taps["ex"].ap(), in_=ex[:])
                                nc.sync.dma_start(out=taps["rl"].ap(), in_=rl[:])
                            for h in range(2):
                                bh = b * 2 + h
                                nc.tensor.matmul(
                                    o1[h][:],
                                    lhsT=vaug[:, bh, kt, 0:65],
                                    rhs=ex[:, h * 512 : (h + 1) * 512],
                                    start=(kt == 0),
                                    stop=(kt == 15),
                                )
                                nc.tensor.matmul(
                                    o2[h][:],
                                    lhsT=vaug[:, bh, kt, 0:64],
                                    rhs=rl[:, h * 512 : (h + 1) * 512],
                                    start=(kt == 0),
                                    stop=False,
                                )
                        for h in range(2):
                            bh, hb = b * 2 + h, h * 64
                            # rank-64 leaky correction accumulates into o2
                            nc.tensor.matmul(
                                o2[h][:],
                                lhsT=ktvs2[hb : hb + 64, b, :],
                                rhs=qkvT[hb : hb + 64, 0, qs : qs + 512],
                                start=False,
                                stop=True,
                            )
                            # epilogue: shift rowsum row to partition 0 (ACT),
                            # broadcast (gpsimd), reciprocal at base 0 (the
                            # custom DVE/gpsimd ops ignore partition offsets)
                            rs = eppool.tile([1, 512], f32, tag="rs")
                            nc.scalar.activation(
                                out=rs[:], in_=o1[h][64:65, :], func=AF.Copy
                            )
                            pbs0 = eppool.tile([64, 512], f32, tag="pbs0")
                            nc.gpsimd.partition_broadcast(pbs0[:], rs[:])
                            pbs = eppool.tile([64, 512], f32, tag="pbs")
                            nc.vector.reciprocal_approx_fast(
                                out=pbs[:], in_=pbs0[:]
                            )
                            t2 = eppool.tile([64, 512], bf, tag="t2")
                            nc.scalar.activation(
                                out=t2[:],
                                in_=o2[h][:],
                                func=AF.Copy,
                                scale=w2_sb[:, h : h + 1],
                            )
                            t1 = eppool.tile([64, 512], bf, tag="t1")
                            nc.vector.tensor_mul(
                                out=t1[:], in0=o1[h][0:64, :], in1=pbs[:]
                            )
                            nc.vector.tensor_add(
                                out=attnT[:, bh, qc * 512 : (qc + 1) * 512],
                                in0=t1[:],
                                in1=t2[:],
                            )
                            if debug_taps and b == 0 and qc == 0 and h == 0:
                                nc.sync.dma_start(out=taps["rc"].ap()[64:65, :], in_=rs[:])
                                nc.sync.dma_start(out=taps["pbs"].ap(), in_=pbs[:])
                                nc.sync.dma_start(out=taps["t1"].ap(), in_=t1[:])
                                nc.sync.dma_start(out=taps["t2"].ap(), in_=t2[:])

            # ---- phase 5: AllToAll + fc_out ----
            for jc in range(8):
                bb, qq = jc // 4, jc % 4
                nc.sync.dma_start(
                    out=a2a_in.ap()[jc],
                    in_=attnT[:, bb * 2 : bb * 2 + 2, qq * 512 : (qq + 1) * 512],
                )
            nc.gpsimd.collective_compute(
                "AllToAll",
                mybir.AluOpType.bypass,
                replica_groups=[list(range(NCORES))],
                ins=[a2a_in.ap().opt()],
                outs=[a2a_out.ap().opt()],
            )
            with tc.tile_pool(name="fcp", bufs=1, space="PSUM") as fcpool:
                fps = {}
                for m in range(4):
                    for n in range(2):
                        fps[m, n] = fcpool.tile(
                            [128, 512], f32, tag=f"fc{m}{n}", name=f"fc{m}{n}"
                        )
                for blk in range(8):
                    g = gpool.tile([128, 512], bf, tag="g")
                    nc.sync.dma_start(out=g[0:64, :], in_=a2a_out.ap()[blk, :, 0, :])
                    nc.sync.dma_start(out=g[64:128, :], in_=a2a_out.ap()[blk, :, 1, :])
                    for hh in range(2):
                        hb = hh * 64
                        for m in range(4):
                            for n in range(2):
                                nc.tensor.matmul(
                                    fps[m, n][:],
                                    lhsT=g[hb : hb + 64, m * 128 : (m + 1) * 128],
                                    rhs=wout_sb[
                                        hb : hb + 64, blk, n * 512 : (n + 1) * 512
                                    ],
                                    start=(blk == 0 and hh == 0),
                                    stop=(blk == 7 and hh == 1),
                                )
                if debug_taps:
                    nc.sync.dma_start(out=taps["qkvT"].ap(), in_=qkvT[:])
                    nc.sync.dma_start(out=taps["vaug"].ap(), in_=vaug[:])
                    nc.sync.dma_start(out=taps["ktok"].ap(), in_=ktok[:])
                    nc.sync.dma_start(out=taps["ktvs"].ap(), in_=ktvs2[:])
                    nc.sync.dma_start(out=taps["attnT"].ap(), in_=attnT[:])
                    nc.sync.dma_start(out=taps["a2a"].ap(), in_=a2a_out.ap())
                for m in range(4):
                    for n in range(2):
                        ob = opool.tile([128, 512], f32, tag="ob")
                        nc.vector.tensor_add(
                            out=ob[:],
                            in0=fps[m, n][:],
                            in1=boutr_sb[:, n * 512 : (n + 1) * 512],
                        )
                        nc.sync.dma_start(
                            out=out_d.ap()[
                                m * 128 : (m + 1) * 128, n * 512 : (n + 1) * 512
                            ],
                            in_=ob[:],
                        )

    nc.compile()
    return nc


def _get_nc():
    global _NC
    if _NC is None:
        _NC = _build()
    return _NC


def _install_ntff_hook():
    """Provide antenv.axon_hooks (absent on this image) so trace=True can
    drive NTFF profiling through the injected libaxon_pjrt.so."""
    import types

    try:
        from antenv.axon_hooks import get_axon_ntff_profile_hook  # noqa: F401

        return
    except ImportError:
        pass
    try:
        import antenv
        from trn_agent_boot.trn_boot import _ntff_profile_via_ctypes

        mod = types.ModuleType("antenv.axon_hooks")
        mod._hook = _ntff_profile_via_ctypes("/opt/axon/libaxon_pjrt.so")
        mod.set_axon_ntff_profile_hook = lambda h: setattr(mod, "_hook", h)
        mod.get_axon_ntff_profile_hook = lambda: mod._hook
        sys.modules["antenv.axon_hooks"] = mod
        antenv.axon_hooks = mod
    except Exception:
        pass


def _make_in_maps(x, Wqkv, W1, W2, Wout, bout):
    x2 = np.asarray(x, np.float32).reshape(T, E)
    xT = np.ascontiguousarray(x2.T).astype(BF16)
    Wq = np.asarray(Wqkv, np.float32)
    wout_b = np.ascontiguousarray(np.asarray(Wout, np.float32)).astype(BF16)
    boutr = np.ascontiguousarray(
        np.broadcast_to(np.asarray(bout, np.float32), (128, E))
    )
    W1f = np.asarray(W1, np.float32).reshape(16)
    W2f = np.asarray(W2, np.float32).reshape(16)

    in_maps = []
    for c in range(NCORES):
        cols = []
        for off, scale in ((0, 0.125), (E, 1.0), (2 * E, 1.0)):
            for h in range(2):
                gh = 2 * c + h
                cols.append(Wq[:, off + gh * 64 : off + (gh + 1) * 64] * scale)
        wqkv_c = np.ascontiguousarray(np.concatenate(cols, axis=1)).astype(BF16)
        w1inv = np.zeros((128, 4, 16), np.float32)
        for b in range(2):
            for h in range(2):
                w1inv[:, b * 2 + h, :] = 1.0 / W1f[2 * c + h]
        w2rep = np.zeros((64, 2), np.float32)
        w2rep[:, 0] = 0.99 * W2f[2 * c]
        w2rep[:, 1] = 0.99 * W2f[2 * c + 1]
        in_maps.append(
            {
                "xT": xT,
                "wqkv": wqkv_c,
                "wout": wout_b,
                "boutr": boutr,
                "w1inv": w1inv.astype(BF16),
                "w2rep": w2rep,
            }
        )
    return in_maps


def kernel(x, Wqkv, W1, W2, Wout, bout):
    from concourse import bass_utils
    from concourse.bass_utils import run_bass_kernel_spmd

    global LAST_EXEC_NS

    _install_ntff_hook()
    # artifact upload needs a bucket this sandbox doesn't have
    bass_utils.upload_artifacts = lambda tmpdir: tmpdir

    nc = _get_nc()
    in_maps = _make_in_maps(x, Wqkv, W1, W2, Wout, bout)

    res = run_bass_kernel_spmd(nc, in_maps, core_ids=list(range(NCORES)), trace=True)
    LAST_EXEC_NS = res.exec_time_ns

    out = np.concatenate(
        [np.asarray(res.results[c]["out"], np.float32) for c in range(NCORES)],
        axis=0,
    )
    return out.reshape(B, S, E)


# revision 23
# speedup vs baseline: 1.5560x; 1.1208x over previous
"""Distributed Trainium2 kernel for the dense_transformer attention block.

Sharding: 16 heads / 8 cores = 2 heads per core (tensor parallel), AllToAll
token-exchange before the output projection, each core computes a 512-token
slice of the final output.

Per-core pipeline (all matmuls bf16, fp32 PSUM accumulation):
  1. qkv^T projection: feats-major  qkvT[128(h0|h1), {q,k,v}, 4096 tok]
     (q columns pre-scaled by 1/sqrt(d) on host)
  2. DMA-transpose v,k tiles to token-major V_aug [128, bh, kt, 64|inv(W1)]
     (col 64 = 1/W1[h] -> the exp@V matmul's 65th row becomes rowsum/W1,
     so its fast-reciprocal is already W1/rowsum)
  3. KtV = K^T V  [64,64] per (b,h)  (rank-64 shortcut for the 0.01*S term
     of leaky_relu: leaky(S)@V = 0.99*relu(S)@V + 0.01*S@V = ... Q@(K^T V))
  4. attention per (b, q-chunk 512), heads packed in PE row groups 0/64:
       S^T[kpos, q(h0|h1)] -> one [128,1024] psum tile per kpos-tile
       exp on ACT, relu on DVE (both heads in one op)
       o1_h[65,512] += V_aug.T @ expS^T   (row 64 = rowsum/W1)
       o2_h[64,512] += V.T @ reluS^T ;  += (0.01/0.99*KtV).T @ qT
       epilogue: rc = recip_fast(o1[64]); gpsimd partition-broadcast;
       attnT[d, bh, s] = o1*rc_bcast + (0.99*W2)*o2
  5. AllToAll(token chunks) -> fc_out: out[512,1024] = sum_heads
     attnT.T @ Wout_rows + bout
"""

import sys

for _p in ("/opt/trn_rl_repo",):
    if _p not in sys.path:
        sys.path.insert(0, _p)

import numpy as np
import ml_dtypes

BF16 = ml_dtypes.bfloat16

E = 1024
T = 4096  # B*S
S = 2048
B = 2
D = 64
NCORES = 8
KCORR = 0.01 / 0.99

LAST_EXEC_NS = None

_NC = None


def _build(debug_taps=False):
    import concourse.bass as bass  # noqa: F401
    import concourse.mybir as mybir
    import concourse.tile as tile
    from concourse import bacc

    bf = mybir.dt.bfloat16
    f32 = mybir.dt.float32
    AF = mybir.ActivationFunctionType

    nc = bacc.Bacc(
        "TRN2",
        target_bir_lowering=False,
        debug=False,
        num_devices=NCORES,
    )

    xT_d = nc.dram_tensor("xT", [E, T], bf, kind="ExternalInput")
    wqkv_d = nc.dram_tensor("wqkv", [E, 384], bf, kind="ExternalInput")
    wout_d = nc.dram_tensor("wout", [E, E], bf, kind="ExternalInput")
    boutr_d = nc.dram_tensor("boutr", [128, E], f32, kind="ExternalInput")
    w1inv_d = nc.dram_tensor("w1inv", [128, 4, 16], bf, kind="ExternalInput")
    w2rep_d = nc.dram_tensor("w2rep", [64, 2], f32, kind="ExternalInput")
    out_d = nc.dram_tensor("out", [512, E], f32, kind="ExternalOutput")
    a2a_in = nc.dram_tensor("a2a_in", [8, 64, 2, 512], bf)
    a2a_out = nc.dram_tensor("a2a_out", [8, 64, 2, 512], bf)

    taps = {}
    if debug_taps:
        taps["qkvT"] = nc.dram_tensor("dbg_qkvT", [128, 3, T], bf, kind="ExternalOutput")
        taps["vaug"] = nc.dram_tensor("dbg_vaug", [128, 4, 16, 128], bf, kind="ExternalOutput")
        taps["ktok"] = nc.dram_tensor("dbg_ktok", [128, 32, 128], bf, kind="ExternalOutput")
        taps["ktvs"] = nc.dram_tensor("dbg_ktvs", [128, 2, 64], bf, kind="ExternalOutput")
        taps["attnT"] = nc.dram_tensor("dbg_attnT", [64, 4, S], bf, kind="ExternalOutput")
        taps["a2a"] = nc.dram_tensor("dbg_a2a", [8, 64, 2, 512], bf, kind="ExternalOutput")
        taps["ex"] = nc.dram_tensor("dbg_ex", [128, 1024], bf, kind="ExternalOutput")
        taps["rl"] = nc.dram_tensor("dbg_rl", [128, 1024], bf, kind="ExternalOutput")
        taps["rc"] = nc.dram_tensor("dbg_rc", [65, 512], f32, kind="ExternalOutput")
        taps["pbs"] = nc.dram_tensor("dbg_pbs", [64, 512], f32, kind="ExternalOutput")
        taps["t1"] = nc.dram_tensor("dbg_t1", [64, 512], bf, kind="ExternalOutput")
        taps["t2"] = nc.dram_tensor("dbg_t2", [64, 512], bf, kind="ExternalOutput")

    with tile.TileContext(nc) as tc:
        with (
            tc.tile_pool(name="const", bufs=1) as cpool,
            tc.tile_pool(name="big", bufs=1) as bigpool,
            tc.tile_pool(name="xin", bufs=4) as xpool,
            tc.tile_pool(name="exr", bufs=3) as expool,
            tc.tile_pool(name="ep", bufs=2) as eppool,
            tc.tile_pool(name="gin", bufs=3) as gpool,
            tc.tile_pool(name="osb", bufs=3) as opool,
        ):
            # ---- constants / persistent tensors ----
            wqkv_sb = cpool.tile([128, 8, 384], bf)
            nc.sync.dma_start(
                out=wqkv_sb[:],
                in_=wqkv_d.ap().rearrange("(kt p) f -> p kt f", p=128),
            )
            wout_sb = cpool.tile([128, 8, E], bf)
            nc.sync.dma_start(
                out=wout_sb[:],
                in_=wout_d.ap().rearrange("(rt p) e -> p rt e", p=128),
            )
            boutr_sb = cpool.tile([128, E], f32)
            nc.sync.dma_start(out=boutr_sb[:], in_=boutr_d.ap())
            w2_sb = cpool.tile([64, 2], f32)
            nc.sync.dma_start(out=w2_sb[:], in_=w2rep_d.ap())

            qkvT = bigpool.tile([128, 3, T], bf)     # [feat(h0|h1), m, tok]
            # inner dim padded to 128 so each kt-tile's transpose destination
            # is 256B-aligned (unaligned DMA-transpose dests corrupt);
            # col 64 = 1/W1[h], cols 65..127 unused
            vaug = bigpool.tile([128, 4, 16, 128], bf)  # [kp, bh, kt, d|w1inv|pad]
            ktok = bigpool.tile([128, 32, 128], bf)  # [kp, tok-tile, feat(h0|h1)]
            attnT = bigpool.tile([64, 4, S], bf)     # [d, bh, s]
            ktvs2 = bigpool.tile([128, 2, 64], bf)   # [di(h0|h1), b, do]

            nc.sync.dma_start(out=vaug[:, :, :, 64:65], in_=w1inv_d.ap())

            # ---- phase 1: qkv^T projection, v/k transposes interleaved ----
            with tc.tile_pool(name="pp", bufs=2, space="PSUM") as ppool:
                for tck in range(8):
                    ps = [
                        ppool.tile([128, 512], f32, tag=f"pm{m}", name=f"pm{m}")
                        for m in range(3)
                    ]
                    xt = xpool.tile([128, 8, 512], bf, tag="xt")
                    nc.sync.dma_start(
                        out=xt[:],
                        in_=xT_d.ap()[
                            :, tck * 512 : (tck + 1) * 512
                        ].rearrange("(kt p) t -> p kt t", p=128),
                    )
                    for kt in range(8):
                        for m in range(3):
                            nc.tensor.matmul(
                                ps[m][:],
                                lhsT=wqkv_sb[:, kt, m * 128 : (m + 1) * 128],
                                rhs=xt[:, kt, :],
                                start=(kt == 0),
                                stop=(kt == 7),
                            )
                    for m in range(3):
                        nc.any.tensor_copy(
                            out=qkvT[:, m, tck * 512 : (tck + 1) * 512],
                            in_=ps[m][:],
                        )
                    # transpose this chunk's v,k tiles to token-major while
                    # the next chunk projects
                    for tt in range(tck * 4, tck * 4 + 4):
                        b, kt = tt // 16, tt % 16
                        tcol = tt * 128
                        for h in range(2):
                            nc.scalar.dma_start(
                                out=vaug[:, b * 2 + h, kt, 0:64],
                                in_=qkvT[h * 64 : h * 64 + 64, 2, tcol : tcol + 128],
                                transpose=True,
                            )
                        nc.sync.dma_start(
                            out=ktok[:, tt, :],
                            in_=qkvT[:, 1, tcol : tcol + 128],
                            transpose=True,
                        )

            # ---- phase 3: KtV rank-64 shortcut ----
            with tc.tile_pool(name="ktvp", bufs=2, space="PSUM") as ktvpool:
                for b in range(2):
                    for h in range(2):
                        bh = b * 2 + h
                        kp = ktvpool.tile([64, 64], f32, tag="ktv")
                        for kt in range(16):
                            nc.tensor.matmul(
                                kp[:],
                                lhsT=ktok[:, b * 16 + kt, h * 64 : h * 64 + 64],
                                rhs=vaug[:, bh, kt, 0:64],
                                start=(kt == 0),
                                stop=(kt == 15),
                            )
                        if h == 0:
                            nc.vector.tensor_scalar_mul(
                                out=ktvs2[0:64, b, :], in0=kp[:], scalar1=KCORR
                            )
                        else:
                            ktmp = eppool.tile([64, 64], bf, tag="ktmp")
                            nc.vector.tensor_scalar_mul(
                                out=ktmp[:], in0=kp[:], scalar1=KCORR
                            )
                            nc.sync.dma_start(
                                out=ktvs2[64:128, b, :], in_=ktmp[:]
                            )

            # ---- phase 4: attention (heads packed in PE row groups) ----
            with (
                tc.tile_pool(name="sgp", bufs=2, space="PSUM") as sgpool,
                tc.tile_pool(name="o1p", bufs=1, space="PSUM") as o1pool,
                tc.tile_pool(name="o2p", bufs=1, space="PSUM") as o2pool,
            ):
                for b in range(2):
                    for qc in range(4):
                        qs = b * S + qc * 512
                        o1 = [
                            o1pool.tile([65, 512], f32, tag=f"o1h{h}", name=f"o1h{h}")
                            for h in range(2)
                        ]
                        o2 = [
                            o2pool.tile([64, 512], f32, tag=f"o2h{h}", name=f"o2h{h}")
                            for h in range(2)
                        ]
                        for kt in range(16):
                            kcol = (b * 16 + kt) * 128
                            sg = sgpool.tile([128, 1024], f32, tag="sg")
                            for h in range(2):
                                hb = h * 64
                                nc.tensor.matmul(
                                    sg[:, h * 512 : (h + 1) * 512],
                                    lhsT=qkvT[hb : hb + 64, 1, kcol : kcol + 128],
                                    rhs=qkvT[hb : hb + 64, 0, qs : qs + 512],
                                    start=True,
                                    stop=True,
                                )
                            ex = expool.tile([128, 1024], bf, tag="ex")
                            rl = expool.tile([128, 1024], bf, tag="rl")
                            nc.scalar.activation(out=ex[:], in_=sg[:], func=AF.Exp)
                            nc.vector.tensor_scalar_max(
                                out=rl[:], in0=sg[:], scalar1=0.0
                            )
                            if debug_taps and b == 0 and qc == 0 and kt == 0:
                                nc.sync.dma_start(out=taps["ex"].ap(), in_=ex[:])
                                nc.sync.dma_start(out=taps["rl"].ap(), in_=rl[:])
                            for h in range(2):
                                bh = b * 2 + h
                                nc.tensor.matmul(
                                    o1[h][:],
                                    lhsT=vaug[:, bh, kt, 0:65],
                                    rhs=ex[:, h * 512 : (h + 1) * 512],
                                    start=(kt == 0),
                                    stop=(kt == 15),
                                )
                                nc.tensor.matmul(
                                    o2[h][:],
                                    lhsT=vaug[:, bh, kt, 0:64],
                                    rhs=rl[:, h * 512 : (h + 1) * 512],
                                    start=(kt == 0),
                                    stop=False,
                                )
                        for h in range(2):
                            bh, hb = b * 2 + h, h * 64
                            # rank-64 leaky correction accumulates into o2
                            nc.tensor.matmul(
                                o2[h][:],
                                lhsT=ktvs2[hb : hb + 64, b, :],
                                rhs=qkvT[hb : hb + 64, 0, qs : qs + 512],
                                start=False,
                                stop=True,
                            )
                            # epilogue: shift rowsum row to partition 0 (ACT),
                            # broadcast (gpsimd), reciprocal at base 0 (the
                            # custom DVE/gpsimd ops ignore partition offsets)
                            rs = eppool.tile([1, 512], f32, tag="rs")
                            nc.scalar.activation(
                                out=rs[:], in_=o1[h][64:65, :], func=AF.Copy
                            )
                            pbs0 = eppool.tile([64, 512], f32, tag="pbs0")
                            nc.gpsimd.partition_broadcast(pbs0[:], rs[:])
                            pbs = eppool.tile([64, 512], f32, tag="pbs")
                            nc.vector.reciprocal_approx_fast(
                                out=pbs[:], in_=pbs0[:]
                            )
                            t2 = eppool.tile([64, 512], bf, tag="t2")
                            nc.scalar.activation(
                                out=t2[:],
                                in_=o2[h][:],
                                func=AF.Copy,
                                scale=w2_sb[:, h : h + 1],
                            )
                            t1 = eppool.tile([64, 512], bf, tag="t1")
                            nc.vector.tensor_mul(
                                out=t1[:], in0=o1[h][0:64, :], in1=pbs[:]
                            )
                            nc.vector.tensor_add(
                                out=attnT[:, bh, qc * 512 : (qc + 1) * 512],
                                in0=t1[:],
                                in1=t2[:],
                            )
                            if debug_taps and b == 0 and qc == 0 and h == 0:
                                nc.sync.dma_start(out=taps["rc"].ap()[64:65, :], in_=rs[:])
                                nc.sync.dma_start(out=taps["pbs"].ap(), in_=pbs[:])
                                nc.sync.dma_start(out=taps["t1"].ap(), in_=t1[:])
                                nc.sync.dma_start(out=taps["t2"].ap(), in_=t2[:])

            # ---- phase 5: AllToAll + fc_out ----
            for jc in range(8):
                bb, qq = jc // 4, jc % 4
                nc.sync.dma_start(
                    out=a2a_in.ap()[jc],
                    in_=attnT[:, bb * 2 : bb * 2 + 2, qq * 512 : (qq + 1) * 512],
                )
            nc.gpsimd.collective_compute(
                "AllToAll",
                mybir.AluOpType.bypass,
                replica_groups=[list(range(NCORES))],
                ins=[a2a_in.ap().opt()],
                outs=[a2a_out.ap().opt()],
            )
            with tc.tile_pool(name="fcp", bufs=1, space="PSUM") as fcpool:
                fps = {}
                for m in range(4):
                    for n in range(2):
                        fps[m, n] = fcpool.tile(
                            [128, 512], f32, tag=f"fc{m}{n}", name=f"fc{m}{n}"
                        )
                for blk in range(8):
                    g = gpool.tile([128, 512], bf, tag="g")
                    nc.sync.dma_start(out=g[0:64, :], in_=a2a_out.ap()[blk, :, 0, :])
                    nc.sync.dma_start(out=g[64:128, :], in_=a2a_out.ap()[blk, :, 1, :])
                    for hh in range(2):
                        hb = hh * 64
                        for m in range(4):
                            for n in range(2):
                                nc.tensor.matmul(
                                    fps[m, n][:],
                                    lhsT=g[hb : hb + 64, m * 128 : (m + 1) * 128],
                                    rhs=wout_sb[
                                        hb : hb + 64, blk, n * 512 : (n + 1) * 512
                                    ],
                                    start=(blk == 0 and hh == 0),
                                    stop=(blk == 7 and hh == 1),
                                )
                if debug_taps:
                    nc.sync.dma_start(out=taps["qkvT"].ap(), in_=qkvT[:])
                    nc.sync.dma_start(out=taps["vaug"].ap(), in_=vaug[:])
                    nc.sync.dma_start(out=taps["ktok"].ap(), in_=ktok[:])
                    nc.sync.dma_start(out=taps["ktvs"].ap(), in_=ktvs2[:])
                    nc.sync.dma_start(out=taps["attnT"].ap(), in_=attnT[:])
                    nc.sync.dma_start(out=taps["a2a"].ap(), in_=a2a_out.ap())
                for m in range(4):
                    for n in range(2):
                        ob = opool.tile([128, 512], f32, tag="ob")
                        nc.vector.tensor_add(
                            out=ob[:],
                            in0=fps[m, n][:],
                            in1=boutr_sb[:, n * 512 : (n + 1) * 512],
                        )
                        nc.sync.dma_start(
                            out=out_d.ap()[
                                m * 128 : (m + 1) * 128, n * 512 : (n + 1) * 512
                            ],
                            in_=ob[:],
                        )

    nc.compile()
    return nc


def _get_nc():
    global _NC
    if _NC is None:
        _NC = _build()
    return _NC


def _install_ntff_hook():
    """Provide antenv.axon_hooks (absent on this image) so trace=True can
    drive NTFF profiling through the injected libaxon_pjrt.so."""
    import types

    try:
        from antenv.axon_hooks import get_axon_ntff_profile_hook  # noqa: F401

        return
    except ImportError:
        pass
    try:
        import antenv
        from trn_agent_boot.trn_boot import _ntff_profile_via_ctypes

        mod = types.ModuleType("antenv.axon_hooks")
        mod._hook = _ntff_profile_via_ctypes("/opt/axon/libaxon_pjrt.so")
        mod.set_axon_ntff_profile_hook = lambda h: setattr(mod, "_hook", h)
        mod.get_axon_ntff_profile_hook = lambda: mod._hook
        sys.modules["antenv.axon_hooks"] = mod
        antenv.axon_hooks = mod
    except Exception:
        pass


def _make_in_maps(x, Wqkv, W1, W2, Wout, bout):
    x2 = np.asarray(x, np.float32).reshape(T, E)
    xT = np.ascontiguousarray(x2.T).astype(BF16)
    Wq = np.asarray(Wqkv, np.float32)
    wout_b = np.ascontiguousarray(np.asarray(Wout, np.float32)).astype(BF16)
    boutr = np.ascontiguousarray(
        np.broadcast_to(np.asarray(bout, np.float32), (128, E))
    )
    W1f = np.asarray(W1, np.float32).reshape(16)
    W2f = np.asarray(W2, np.float32).reshape(16)

    in_maps = []
    for c in range(NCORES):
        cols = []
        for off, scale in ((0, 0.125), (E, 1.0), (2 * E, 1.0)):
            for h in range(2):
                gh = 2 * c + h
                cols.append(Wq[:, off + gh * 64 : off + (gh + 1) * 64] * scale)
        wqkv_c = np.ascontiguousarray(np.concatenate(cols, axis=1)).astype(BF16)
        w1inv = np.zeros((128, 4, 16), np.float32)
        for b in range(2):
            for h in range(2):
                w1inv[:, b * 2 + h, :] = 1.0 / W1f[2 * c + h]
        w2rep = np.zeros((64, 2), np.float32)
        w2rep[:, 0] = 0.99 * W2f[2 * c]
        w2rep[:, 1] = 0.99 * W2f[2 * c + 1]
        in_maps.append(
            {
                "xT": xT,
                "wqkv": wqkv_c,
                "wout": wout_b,
                "boutr": boutr,
                "w1inv": w1inv.astype(BF16),
                "w2rep": w2rep,
            }
        )
    return in_maps


def kernel(x, Wqkv, W1, W2, Wout, bout):
    from concourse import bass_utils
    from concourse.bass_utils import run_bass_kernel_spmd

    global LAST_EXEC_NS

    _install_ntff_hook()
    # artifact upload needs a bucket this sandbox doesn't have
    bass_utils.upload_artifacts = lambda tmpdir: tmpdir

    nc = _get_nc()
    in_maps = _make_in_maps(x, Wqkv, W1, W2, Wout, bout)

    res = run_bass_kernel_spmd(nc, in_maps, core_ids=list(range(NCORES)), trace=True)
    LAST_EXEC_NS = res.exec_time_ns

    out = np.concatenate(
        [np.asarray(res.results[c]["out"], np.float32) for c in range(NCORES)],
        axis=0,
    )
    return out.reshape(B, S, E)


# revision 25
# speedup vs baseline: 1.8282x; 1.1749x over previous
"""Distributed Trainium2 kernel for the dense_transformer attention block.

Sharding: 16 heads / 8 cores = 2 heads per core (tensor parallel), AllToAll
token-exchange before the output projection, each core computes a 512-token
slice of the final output.

Per-core pipeline (all matmuls bf16, fp32 PSUM accumulation):
  1. qkv^T projection: feats-major  qkvT[128(h0|h1), {q,k,v}, 4096 tok]
     (q columns pre-scaled by 1/sqrt(d) on host)
  2. DMA-transpose v,k tiles to token-major V_aug [128, bh, kt, 64|inv(W1)]
     (col 64 = 1/W1[h] -> the exp@V matmul's 65th row becomes rowsum/W1,
     so its fast-reciprocal is already W1/rowsum)
  3. KtV = K^T V  [64,64] per (b,h)  (rank-64 shortcut for the 0.01*S term
     of leaky_relu: leaky(S)@V = 0.99*relu(S)@V + 0.01*S@V = ... Q@(K^T V))
  4. attention per (b, q-chunk 512), heads packed in PE row groups 0/64:
       S^T[kpos, q(h0|h1)] -> one [128,1024] psum tile per kpos-tile
       exp on ACT, relu on DVE (both heads in one op)
       o1_h[65,512] += V_aug.T @ expS^T   (row 64 = rowsum/W1)
       o2_h[64,512] += V.T @ reluS^T ;  += (0.01/0.99*KtV).T @ qT
       epilogue: rc = recip_fast(o1[64]); gpsimd partition-broadcast;
       attnT[d, bh, s] = o1*rc_bcast + (0.99*W2)*o2
  5. AllToAll(token chunks) -> fc_out: out[512,1024] = sum_heads
     attnT.T @ Wout_rows + bout
"""

import sys

for _p in ("/opt/trn_rl_repo",):
    if _p not in sys.path:
        sys.path.insert(0, _p)

import numpy as np
import ml_dtypes

BF16 = ml_dtypes.bfloat16

E = 1024
T = 4096  # B*S
S = 2048
B = 2
D = 64
NCORES = 8
KCORR = 0.01 / 0.99

LAST_EXEC_NS = None

_NC = None


def _build(debug_taps=False):
    import concourse.bass as bass  # noqa: F401
    import concourse.mybir as mybir
    import concourse.tile as tile
    from concourse import bacc

    bf = mybir.dt.bfloat16
    f32 = mybir.dt.float32
    AF = mybir.ActivationFunctionType

    nc = bacc.Bacc(
        "TRN2",
        target_bir_lowering=False,
        debug=False,
        num_devices=NCORES,
    )

    xT_d = nc.dram_tensor("xT", [E, T], bf, kind="ExternalInput")
    wqkv_d = nc.dram_tensor("wqkv", [E, 384], bf, kind="ExternalInput")
    wout_d = nc.dram_tensor("wout", [E, E], bf, kind="ExternalInput")
    boutr_d = nc.dram_tensor("boutr", [128, E], f32, kind="ExternalInput")
    w1inv_d = nc.dram_tensor("w1inv", [128, 4, 16], bf, kind="ExternalInput")
    w2rep_d = nc.dram_tensor("w2rep", [64, 2], f32, kind="ExternalInput")
    out_d = nc.dram_tensor("out", [512, E], f32, kind="ExternalOutput")
    a2a_in = nc.dram_tensor("a2a_in", [8, 64, 2, 512], bf)
    a2a_out = nc.dram_tensor("a2a_out", [8, 64, 2, 512], bf)

    taps = {}
    if debug_taps:
        taps["qkvT"] = nc.dram_tensor("dbg_qkvT", [128, 3, T], bf, kind="ExternalOutput")
        taps["vaug"] = nc.dram_tensor("dbg_vaug", [128, 4, 16, 128], bf, kind="ExternalOutput")
        taps["ktok"] = nc.dram_tensor("dbg_ktok", [128, 32, 128], bf, kind="ExternalOutput")
        taps["ktvs"] = nc.dram_tensor("dbg_ktvs", [128, 2, 64], bf, kind="ExternalOutput")
        taps["attnT"] = nc.dram_tensor("dbg_attnT", [64, 4, S], bf, kind="ExternalOutput")
        taps["a2a"] = nc.dram_tensor("dbg_a2a", [8, 64, 2, 512], bf, kind="ExternalOutput")
        taps["ex"] = nc.dram_tensor("dbg_ex", [128, 1024], bf, kind="ExternalOutput")
        taps["rl"] = nc.dram_tensor("dbg_rl", [128, 1024], bf, kind="ExternalOutput")
        taps["rc"] = nc.dram_tensor("dbg_rc", [65, 512], f32, kind="ExternalOutput")
        taps["pbs"] = nc.dram_tensor("dbg_pbs", [64, 512], f32, kind="ExternalOutput")
        taps["t1"] = nc.dram_tensor("dbg_t1", [64, 512], bf, kind="ExternalOutput")
        taps["t2"] = nc.dram_tensor("dbg_t2", [64, 512], bf, kind="ExternalOutput")

    with tile.TileContext(nc) as tc:
        with (
            tc.tile_pool(name="const", bufs=1) as cpool,
            tc.tile_pool(name="big", bufs=1) as bigpool,
            tc.tile_pool(name="xin", bufs=4) as xpool,
            tc.tile_pool(name="exr", bufs=3) as expool,
            tc.tile_pool(name="ep", bufs=2) as eppool,
            tc.tile_pool(name="gin", bufs=3) as gpool,
            tc.tile_pool(name="osb", bufs=3) as opool,
        ):
            # ---- constants / persistent tensors ----
            wqkv_sb = cpool.tile([128, 8, 384], bf)
            nc.sync.dma_start(
                out=wqkv_sb[:],
                in_=wqkv_d.ap().rearrange("(kt p) f -> p kt f", p=128),
            )
            wout_sb = cpool.tile([128, 8, E], bf)
            nc.sync.dma_start(
                out=wout_sb[:],
                in_=wout_d.ap().rearrange("(rt p) e -> p rt e", p=128),
            )
            boutr_sb = cpool.tile([128, E], f32)
            nc.sync.dma_start(out=boutr_sb[:], in_=boutr_d.ap())
            w2_sb = cpool.tile([64, 2], f32)
            nc.sync.dma_start(out=w2_sb[:], in_=w2rep_d.ap())

            qkvT = bigpool.tile([128, 3, T], bf)     # [feat(h0|h1), m, tok]
            # inner dim padded to 128 so each kt-tile's transpose destination
            # is 256B-aligned (unaligned DMA-transpose dests corrupt);
            # col 64 = 1/W1[h], cols 65..127 unused
            vaug = bigpool.tile([128, 4, 16, 128], bf)  # [kp, bh, kt, d|w1inv|pad]
            ktok = bigpool.tile([128, 32, 128], bf)  # [kp, tok-tile, feat(h0|h1)]
            attnT = bigpool.tile([64, 4, S], bf)     # [d, bh, s]
            ktvs2 = bigpool.tile([128, 2, 64], bf)   # [di(h0|h1), b, do]

            nc.sync.dma_start(out=vaug[:, :, :, 64:65], in_=w1inv_d.ap())

            # ---- phase 1: qkv^T projection, v/k transposes interleaved ----
            with tc.tile_pool(name="pp", bufs=2, space="PSUM") as ppool:
                for tck in range(8):
                    ps = [
                        ppool.tile([128, 512], f32, tag=f"pm{m}", name=f"pm{m}")
                        for m in range(3)
                    ]
                    xt = xpool.tile([128, 8, 512], bf, tag="xt")
                    nc.sync.dma_start(
                        out=xt[:],
                        in_=xT_d.ap()[
                            :, tck * 512 : (tck + 1) * 512
                        ].rearrange("(kt p) t -> p kt t", p=128),
                    )
                    for kt in range(8):
                        for m in range(3):
                            nc.tensor.matmul(
                                ps[m][:],
                                lhsT=wqkv_sb[:, kt, m * 128 : (m + 1) * 128],
                                rhs=xt[:, kt, :],
                                start=(kt == 0),
                                stop=(kt == 7),
                            )
                    for m in range(3):
                        nc.vector.tensor_copy(
                            out=qkvT[:, m, tck * 512 : (tck + 1) * 512],
                            in_=ps[m][:],
                        )
                    # transpose this chunk's v,k tiles to token-major while
                    # the next chunk projects; alternate HWDGE queues
                    tq = [nc.scalar, nc.sync]
                    qi = 0
                    for tt in range(tck * 4, tck * 4 + 4):
                        b, kt = tt // 16, tt % 16
                        tcol = tt * 128
                        for h in range(2):
                            tq[qi % 2].dma_start(
                                out=vaug[:, b * 2 + h, kt, 0:64],
                                in_=qkvT[h * 64 : h * 64 + 64, 2, tcol : tcol + 128],
                                transpose=True,
                            )
                            qi += 1
                        tq[qi % 2].dma_start(
                            out=ktok[:, tt, :],
                            in_=qkvT[:, 1, tcol : tcol + 128],
                            transpose=True,
                        )
                        qi += 1

            # ---- phase 3: KtV rank-64 shortcut ----
            with tc.tile_pool(name="ktvp", bufs=2, space="PSUM") as ktvpool:
                for b in range(2):
                    for h in range(2):
                        bh = b * 2 + h
                        kp = ktvpool.tile([64, 64], f32, tag="ktv")
                        for kt in range(16):
                            nc.tensor.matmul(
                                kp[:],
                                lhsT=ktok[:, b * 16 + kt, h * 64 : h * 64 + 64],
                                rhs=vaug[:, bh, kt, 0:64],
                                start=(kt == 0),
                                stop=(kt == 15),
                            )
                        if h == 0:
                            nc.vector.tensor_scalar_mul(
                                out=ktvs2[0:64, b, :], in0=kp[:], scalar1=KCORR
                            )
                        else:
                            ktmp = eppool.tile([64, 64], bf, tag="ktmp")
                            nc.vector.tensor_scalar_mul(
                                out=ktmp[:], in0=kp[:], scalar1=KCORR
                            )
                            nc.sync.dma_start(
                                out=ktvs2[64:128, b, :], in_=ktmp[:]
                            )

            # ---- phase 4: attention (heads packed in PE row groups) ----
            with (
                tc.tile_pool(name="sgp", bufs=2, space="PSUM") as sgpool,
                tc.tile_pool(name="o1p", bufs=1, space="PSUM") as o1pool,
                tc.tile_pool(name="o2p", bufs=1, space="PSUM") as o2pool,
            ):
                for b in range(2):
                    for qc in range(4):
                        qs = b * S + qc * 512
                        o1 = [
                            o1pool.tile([65, 512], f32, tag=f"o1h{h}", name=f"o1h{h}")
                            for h in range(2)
                        ]
                        o2 = [
                            o2pool.tile([64, 512], f32, tag=f"o2h{h}", name=f"o2h{h}")
                            for h in range(2)
                        ]
                        for kt in range(16):
                            kcol = (b * 16 + kt) * 128
                            sg = sgpool.tile([128, 1024], f32, tag="sg")
                            for h in range(2):
                                hb = h * 64
                                nc.tensor.matmul(
                                    sg[:, h * 512 : (h + 1) * 512],
                                    lhsT=qkvT[hb : hb + 64, 1, kcol : kcol + 128],
                                    rhs=qkvT[hb : hb + 64, 0, qs : qs + 512],
                                    start=True,
                                    stop=True,
                                )
                            ex = expool.tile([128, 1024], bf, tag="ex")
                            rl = expool.tile([128, 1024], bf, tag="rl")
                            nc.scalar.activation(out=ex[:], in_=sg[:], func=AF.Exp)
                            nc.vector.tensor_scalar_max(
                                out=rl[:], in0=sg[:], scalar1=0.0
                            )
                            if debug_taps and b == 0 and qc == 0 and kt == 0:
                                nc.sync.dma_start(out=taps["ex"].ap(), in_=ex[:])
                                nc.sync.dma_start(out=taps["rl"].ap(), in_=rl[:])
                            for h in range(2):
                                bh = b * 2 + h
                                nc.tensor.matmul(
                                    o1[h][:],
                                    lhsT=vaug[:, bh, kt, 0:65],
                                    rhs=ex[:, h * 512 : (h + 1) * 512],
                                    start=(kt == 0),
                                    stop=(kt == 15),
                                )
                                nc.tensor.matmul(
                                    o2[h][:],
                                    lhsT=vaug[:, bh, kt, 0:64],
                                    rhs=rl[:, h * 512 : (h + 1) * 512],
                                    start=(kt == 0),
                                    stop=False,
                                )
                        for h in range(2):
                            bh, hb = b * 2 + h, h * 64
                            # rank-64 leaky correction accumulates into o2
                            nc.tensor.matmul(
                                o2[h][:],
                                lhsT=ktvs2[hb : hb + 64, b, :],
                                rhs=qkvT[hb : hb + 64, 0, qs : qs + 512],
                                start=False,
                                stop=True,
                            )
                            # epilogue: shift rowsum row to partition 0 (ACT),
                            # broadcast (gpsimd), reciprocal at base 0 (the
                            # custom DVE/gpsimd ops ignore partition offsets)
                            rs = eppool.tile([1, 512], f32, tag="rs")
                            nc.scalar.activation(
                                out=rs[:], in_=o1[h][64:65, :], func=AF.Copy
                            )
                            pbs0 = eppool.tile([64, 512], f32, tag="pbs0")
                            nc.gpsimd.partition_broadcast(pbs0[:], rs[:])
                            pbs = eppool.tile([64, 512], f32, tag="pbs")
                            nc.vector.reciprocal_approx_fast(
                                out=pbs[:], in_=pbs0[:]
                            )
                            t2 = eppool.tile([64, 512], bf, tag="t2")
                            nc.scalar.activation(
                                out=t2[:],
                                in_=o2[h][:],
                                func=AF.Copy,
                                scale=w2_sb[:, h : h + 1],
                            )
                            t1 = eppool.tile([64, 512], bf, tag="t1")
                            nc.vector.tensor_mul(
                                out=t1[:], in0=o1[h][0:64, :], in1=pbs[:]
                            )
                            nc.vector.tensor_add(
                                out=attnT[:, bh, qc * 512 : (qc + 1) * 512],
                                in0=t1[:],
                                in1=t2[:],
                            )
                            if debug_taps and b == 0 and qc == 0 and h == 0:
                                nc.sync.dma_start(out=taps["rc"].ap()[64:65, :], in_=rs[:])
                                nc.sync.dma_start(out=taps["pbs"].ap(), in_=pbs[:])
                                nc.sync.dma_start(out=taps["t1"].ap(), in_=t1[:])
                                nc.sync.dma_start(out=taps["t2"].ap(), in_=t2[:])

            # ---- phase 5: AllToAll + fc_out ----
            for jc in range(8):
                bb, qq = jc // 4, jc % 4
                nc.sync.dma_start(
                    out=a2a_in.ap()[jc],
                    in_=attnT[:, bb * 2 : bb * 2 + 2, qq * 512 : (qq + 1) * 512],
                )
            nc.gpsimd.collective_compute(
                "AllToAll",
                mybir.AluOpType.bypass,
                replica_groups=[list(range(NCORES))],
                ins=[a2a_in.ap().opt()],
                outs=[a2a_out.ap().opt()],
            )
            with tc.tile_pool(name="fcp", bufs=1, space="PSUM") as fcpool:
                fps = {}
                for m in range(4):
                    for n in range(2):
                        fps[m, n] = fcpool.tile(
                            [128, 512], f32, tag=f"fc{m}{n}", name=f"fc{m}{n}"
                        )
                for blk in range(8):
                    g = gpool.tile([128, 512], bf, tag="g")
                    nc.gpsimd.dma_start(out=g[0:64, :], in_=a2a_out.ap()[blk, :, 0, :])
                    nc.gpsimd.dma_start(out=g[64:128, :], in_=a2a_out.ap()[blk, :, 1, :])
                    # g stacks both heads on 128 partitions -> full K=128
                    for m in range(4):
                        for n in range(2):
                            nc.tensor.matmul(
                                fps[m, n][:],
                                lhsT=g[:, m * 128 : (m + 1) * 128],
                                rhs=wout_sb[:, blk, n * 512 : (n + 1) * 512],
                                start=(blk == 0),
                                stop=(blk == 7),
                            )
                if debug_taps:
                    nc.sync.dma_start(out=taps["qkvT"].ap(), in_=qkvT[:])
                    nc.sync.dma_start(out=taps["vaug"].ap(), in_=vaug[:])
                    nc.sync.dma_start(out=taps["ktok"].ap(), in_=ktok[:])
                    nc.sync.dma_start(out=taps["ktvs"].ap(), in_=ktvs2[:])
                    nc.sync.dma_start(out=taps["attnT"].ap(), in_=attnT[:])
                    nc.sync.dma_start(out=taps["a2a"].ap(), in_=a2a_out.ap())
                for m in range(4):
                    for n in range(2):
                        ob = opool.tile([128, 512], f32, tag="ob")
                        nc.vector.tensor_add(
                            out=ob[:],
                            in0=fps[m, n][:],
                            in1=boutr_sb[:, n * 512 : (n + 1) * 512],
                        )
                        nc.sync.dma_start(
                            out=out_d.ap()[
                                m * 128 : (m + 1) * 128, n * 512 : (n + 1) * 512
                            ],
                            in_=ob[:],
                        )

    nc.compile()
    return nc


def _get_nc():
    global _NC
    if _NC is None:
        _NC = _build()
    return _NC


def _install_ntff_hook():
    """Provide antenv.axon_hooks (absent on this image) so trace=True can
    drive NTFF profiling through the injected libaxon_pjrt.so."""
    import types

    try:
        from antenv.axon_hooks import get_axon_ntff_profile_hook  # noqa: F401

        return
    except ImportError:
        pass
    try:
        import antenv
        from trn_agent_boot.trn_boot import _ntff_profile_via_ctypes

        mod = types.ModuleType("antenv.axon_hooks")
        mod._hook = _ntff_profile_via_ctypes("/opt/axon/libaxon_pjrt.so")
        mod.set_axon_ntff_profile_hook = lambda h: setattr(mod, "_hook", h)
        mod.get_axon_ntff_profile_hook = lambda: mod._hook
        sys.modules["antenv.axon_hooks"] = mod
        antenv.axon_hooks = mod
    except Exception:
        pass


def _make_in_maps(x, Wqkv, W1, W2, Wout, bout):
    x2 = np.asarray(x, np.float32).reshape(T, E)
    xT = np.ascontiguousarray(x2.T).astype(BF16)
    Wq = np.asarray(Wqkv, np.float32)
    wout_b = np.ascontiguousarray(np.asarray(Wout, np.float32)).astype(BF16)
    boutr = np.ascontiguousarray(
        np.broadcast_to(np.asarray(bout, np.float32), (128, E))
    )
    W1f = np.asarray(W1, np.float32).reshape(16)
    W2f = np.asarray(W2, np.float32).reshape(16)

    in_maps = []
    for c in range(NCORES):
        cols = []
        for off, scale in ((0, 0.125), (E, 1.0), (2 * E, 1.0)):
            for h in range(2):
                gh = 2 * c + h
                cols.append(Wq[:, off + gh * 64 : off + (gh + 1) * 64] * scale)
        wqkv_c = np.ascontiguousarray(np.concatenate(cols, axis=1)).astype(BF16)
        w1inv = np.zeros((128, 4, 16), np.float32)
        for b in range(2):
            for h in range(2):
                w1inv[:, b * 2 + h, :] = 1.0 / W1f[2 * c + h]
        w2rep = np.zeros((64, 2), np.float32)
        w2rep[:, 0] = 0.99 * W2f[2 * c]
        w2rep[:, 1] = 0.99 * W2f[2 * c + 1]
        in_maps.append(
            {
                "xT": xT,
                "wqkv": wqkv_c,
                "wout": wout_b,
                "boutr": boutr,
                "w1inv": w1inv.astype(BF16),
                "w2rep": w2rep,
            }
        )
    return in_maps


def kernel(x, Wqkv, W1, W2, Wout, bout):
    from concourse import bass_utils
    from concourse.bass_utils import run_bass_kernel_spmd

    global LAST_EXEC_NS

    _install_ntff_hook()
    # artifact upload needs a bucket this sandbox doesn't have
    bass_utils.upload_artifacts = lambda tmpdir: tmpdir

    nc = _get_nc()
    in_maps = _make_in_maps(x, Wqkv, W1, W2, Wout, bout)

    res = run_bass_kernel_spmd(nc, in_maps, core_ids=list(range(NCORES)), trace=True)
    LAST_EXEC_NS = res.exec_time_ns

    out = np.concatenate(
        [np.asarray(res.results[c]["out"], np.float32) for c in range(NCORES)],
        axis=0,
    )
    return out.reshape(B, S, E)
